# revision 35
# baseline (speedup 1.0000x reference)
"""Causal self-attention (L=8192, D=2048) on 8 TRN2 NeuronCores.

Sharding: core c owns query rows x[c::8] (stride-8 interleave); KV rows
[c*1024, (c+1)*1024) are projected locally.  Local q-tile p (128 rows) covers
global rows [1024p + c, 1024p + 1016 + c], so causally it needs exactly KV
j-tiles 0..8p+7 - identical on every core (load-balanced static SPMD).

Precision: keys >= 1024 are consumed through fp8-e4m3 K/V/P with DoubleRow
matmuls (2x PE rate); keys < 1024 (where early rows' softmax is concentrated
and quantization noise would not average out) stay bf16.  Every rank packs its
K^T/V/ones j-tiles in fp8 ([K8 2048 | V8 2048 | ones 8 | pad] = 4112B/row) and
AllGathers them in two halves; rank 0's bf16 pack ([Kbf | Vbf | ones] x4112
bf16 cols) is broadcast via a rank-masked AllReduce(add).  exp is computed as
exp(s/sqrt(d) - 2.5) so P fits fp8 range; the shift cancels in num/den.

Host-side prep (free): x^T/z^T and all weight panels are pre-transposed and
pre-cast to bf16 in DMA-ready layouts, so phase 1 is pure projection matmuls.
A ~96-matmul warmup burst trips the PE HAM clock gate to 2.4 GHz before the
first projection.

Phase 1: warmup -> K proj -> V(j-tiles 0-3) -> AG8-A -> V(4-7) -> AG8-B + AR
-> Q proj (bf16 + fp8 sinks).  Phase 2 runs two q-group passes (q-tiles 0-3,
then 4-7) so only 4 f32 accumulators are SBUF-resident; within a pass, fp8
windows r>=1 run S^T (DoubleRow over dt pairs) -> exp -> P^T@[V|1] (DoubleRow
over k-tile pairs), and the two r=0 windows run the bf16 path from the
AllReduced pack.  Per-q-tile epilogue (scale by 1/den, +bv, DMA out) issues as
soon as that q-tile's last window is accumulated.
"""

import math
import time
from contextlib import ExitStack

import ml_dtypes
import numpy as np

import concourse.bass as bass
import concourse.tile as tile
from concourse import bacc, mybir
from concourse.bass_utils import run_bass_kernel_spmd

L = 8192
D = 2048  # d_x == d_attn == d_v
NCORES = 8
NDT = D // 128  # 16 contraction tiles
NQT = 8  # local 128-row q-tiles per core
PACK = 4112  # fp8: 2048 K | 2048 V | 8 ones | 8 pad ; bf16 pack same col count
V_OFF = 2048
ONES_OFF = 4096
SCALE = 1.0 / math.sqrt(D)
SHIFT = 2.5  # exp(s*SCALE - SHIFT): max p ~ e^3 = 20 << 240 (fp8e4 max)

F32 = mybir.dt.float32
BF16 = mybir.dt.bfloat16
F8 = mybir.dt.float8e4
DR = mybir.MatmulPerfMode.DoubleRow
Ident = mybir.ActivationFunctionType.Identity
Copy = mybir.ActivationFunctionType.Copy
Exp = mybir.ActivationFunctionType.Exp

_cache = {}


def _build():
    nc = bacc.Bacc("TRN2", num_devices=NCORES)

    zt_d = nc.dram_tensor("zt", [128, NDT, 1024], BF16, kind="ExternalInput")
    zt8_d = nc.dram_tensor("zt8", [128, NDT, 1024], F8, kind="ExternalInput")
    xt8_d = nc.dram_tensor("xt8", [128, NDT, 1024], F8, kind="ExternalInput")
    xtb_d = nc.dram_tensor("xtb", [128, NDT, 128], BF16, kind="ExternalInput")
    wkp_d = nc.dram_tensor("wkp", [NDT, 128, NDT, 128], BF16, kind="ExternalInput")
    wqp_d = nc.dram_tensor("wqp", [NDT, 128, NDT, 128], BF16, kind="ExternalInput")
    wq8_d = nc.dram_tensor("wq8", [NDT, 128, NDT, 128], F8, kind="ExternalInput")
    wvt_d = nc.dram_tensor("wvt", [128, NDT, D], BF16, kind="ExternalInput")
    wv8_d = nc.dram_tensor("wv8", [128, NDT, D], F8, kind="ExternalInput")
    bq_d = nc.dram_tensor("bq", [D], F32, kind="ExternalInput")
    bk_d = nc.dram_tensor("bk", [D], F32, kind="ExternalInput")
    bv_d = nc.dram_tensor("bv", [D], F32, kind="ExternalInput")
    iu_d = nc.dram_tensor("iu", [128], F32, kind="ExternalInput")
    rk0_d = nc.dram_tensor("rk0", [8], F32, kind="ExternalInput")
    out_d = nc.dram_tensor("out", [1024, D], F32, kind="ExternalOutput")

    kv8_loc = [nc.dram_tensor(f"kv8loc{h}", [4, 128, PACK], F8) for h in range(2)]
    kv8_g = [
        nc.dram_tensor(f"kv8g{h}", [32, 128, PACK], F8, addr_space="Shared")
        for h in range(2)
    ]
    # bf16 pack only carries rank 0's keys 0-255 (j-tiles 0-1)
    kvbf_loc = nc.dram_tensor("kvbfloc", [2, 128, PACK], BF16)
    kvbf_g = nc.dram_tensor("kvbfg", [2, 128, PACK], BF16, addr_space="Shared")

    groups = [list(range(NCORES))]

    with tile.TileContext(nc) as tc:
        with ExitStack() as outer:
            cp = outer.enter_context(tc.tile_pool(name="consts", bufs=1))
            # jg[v, r] = 128*r + v
            jg = cp.tile([128, 8], F32, tag="jg")
            nc.gpsimd.iota(
                jg,
                pattern=[[128, 8]],
                base=0,
                channel_multiplier=1,
                allow_small_or_imprecise_dtypes=True,
            )
            # iu_bc[v, u] = 8*u + c (same for all partitions v)
            iu_bc = cp.tile([128, 128], F32, tag="iu_bc")
            nc.gpsimd.dma_start(
                iu_bc, bass.AP(tensor=iu_d, offset=0, ap=[[0, 128], [1, 128]])
            )
            # rk0f[v, j] = 1.0 iff this core is rank 0
            rk0f = cp.tile([128, 8], F32, tag="rk0f")
            nc.gpsimd.dma_start(
                rk0f, bass.AP(tensor=rk0_d, offset=0, ap=[[0, 128], [1, 8]])
            )
            rk0_sc = rk0f[:, 0:1]
            ones_bf = cp.tile([128, 8], BF16, tag="ones_bf")  # rk0-masked ones
            nc.vector.tensor_copy(ones_bf, rk0f)
            ones8 = cp.tile([128, 8], F8, tag="ones8")
            nc.vector.memset(ones8, 1.0)
            # msk[m][v, u] = (8u + c >= 128m + v): causal mask of diagonal tile
            msk = []
            for m in range(8):
                mt = cp.tile([128, 128], BF16, tag=f"msk{m}")
                nc.vector.tensor_scalar(
                    mt, iu_bc, jg[:, m : m + 1], None, mybir.AluOpType.is_ge
                )
                msk.append(mt)
            bq_sb = cp.tile([128, NDT], F32, tag="bq")
            nc.gpsimd.dma_start(
                bq_sb, bass.AP(tensor=bq_d, offset=0, ap=[[1, 128], [128, NDT]])
            )
            bk_sb = cp.tile([128, NDT], F32, tag="bk")
            nc.gpsimd.dma_start(
                bk_sb, bass.AP(tensor=bk_d, offset=0, ap=[[1, 128], [128, NDT]])
            )
            bkm_sb = cp.tile([128, NDT], F32, tag="bkm")  # rk0-masked K bias
            nc.vector.tensor_scalar_mul(bkm_sb, bk_sb, rk0_sc)
            nshift = cp.tile([128, 1], F32, tag="nshift")
            nc.vector.memset(nshift, -SHIFT)

            # qt/qt8 persist from Q projection through all of phase 2
            qtp = outer.enter_context(tc.tile_pool(name="qt", bufs=1))
            qt = qtp.tile([128, NDT, 1024], BF16, tag="qt")
            qt8 = qtp.tile([128, NDT, 1024], F8, tag="qt8")

            # ---- PE warmup: trip the HAM clock gate before real matmuls ----
            with ExitStack() as wm:
                wmp = wm.enter_context(tc.tile_pool(name="wm", bufs=1))
                wrm = wmp.tile([128, 128], BF16, tag="wrm")
                nc.vector.memset(wrm, 0.5)
                wps = wm.enter_context(tc.tile_pool(name="wm_ps", bufs=2, space="PSUM"))
                for _ in range(36):
                    wp_ps = wps.tile([128, 128], F32, tag="wps")
                    nc.tensor.matmul(wp_ps, wrm, wrm, start=True, stop=True)

            # ---------------- Phase 1: projections + collectives ----------------
            with ExitStack() as p1:
                ztp = p1.enter_context(tc.tile_pool(name="zt", bufs=1))
                zt = ztp.tile([128, NDT, 1024], BF16, tag="zt")
                wvbfp = p1.enter_context(tc.tile_pool(name="wvbf", bufs=1))
                wpp = p1.enter_context(tc.tile_pool(name="wp", bufs=2))
                w8pp = p1.enter_context(tc.tile_pool(name="w8p", bufs=2))
                stg = p1.enter_context(tc.tile_pool(name="stg", bufs=2))
                inp = p1.enter_context(tc.tile_pool(name="inp", bufs=1))
                kqps = p1.enter_context(
                    tc.tile_pool(name="kq_ps", bufs=2, space="PSUM")
                )

                # critical first loads on the sync ring (first delivery ~3us;
                # the scalar HWDGE takes ~40us to first delivery)
                wk_pre = [
                    wpp.tile([128, NDT, 128], BF16, tag="wp", name=f"wk_pre{t}")
                    for t in range(2)
                ]
                for t in range(2):
                    nc.sync.dma_start(wk_pre[t], wkp_d[t])
                for ch in range(4):
                    nc.sync.dma_start(
                        zt[:, 4 * ch : 4 * ch + 4, :], zt_d[:, 4 * ch : 4 * ch + 4, :]
                    )

                # ---- K projection (bf16) ----
                for t in range(NDT):
                    if t < 2:
                        wpt = wk_pre[t]
                    else:
                        # early panels on the sync ring (scalar HWDGE is still
                        # ramping for the first ~40us)
                        wpt = wpp.tile([128, NDT, 128], BF16, tag="wp")
                        (nc.sync if t < 5 else nc.scalar).dma_start(wpt, wkp_d[t])
                    ps0 = kqps.tile([128, 512], F32, tag="ps")
                    ps1 = kqps.tile([128, 512], F32, tag="ps")
                    for dt in range(NDT):
                        nc.tensor.matmul(
                            ps0,
                            wpt[:, dt, :],
                            zt[:, dt, 0:512],
                            start=(dt == 0),
                            stop=(dt == NDT - 1),
                        )
                        nc.tensor.matmul(
                            ps1,
                            wpt[:, dt, :],
                            zt[:, dt, 512:1024],
                            start=(dt == 0),
                            stop=(dt == NDT - 1),
                        )
                    for half, ps in ((0, ps0), (1, ps1)):
                        k8 = stg.tile([128, 512], F8, tag="k8")
                        nc.scalar.activation(k8, ps, Ident, bias=bk_sb[:, t : t + 1])
                        for q in range(4):
                            nc.sync.dma_start(
                                kv8_loc[half][q][:, t * 128 : (t + 1) * 128],
                                k8[:, q * 128 : (q + 1) * 128],
                            )
                        if half == 0:  # bf16 pack: keys 0-255 only
                            kb = stg.tile([128, 256], BF16, tag="kb")
                            nc.scalar.activation(
                                kb,
                                ps[:, 0:256],
                                Ident,
                                bias=bkm_sb[:, t : t + 1],
                                scale=rk0_sc,
                            )
                            for q in range(2):
                                nc.sync.dma_start(
                                    kvbf_loc[q][:, t * 128 : (t + 1) * 128],
                                    kb[:, q * 128 : (q + 1) * 128],
                                )

                # bulk loads, ordered by first use (scalar ring drains in order)
                wvbf0 = wvbfp.tile([128, NDT, 1024], BF16, tag="wvbf", name="wvbf0")
                nc.scalar.dma_start(wvbf0, wvt_d[:, :, 0:1024])
                wv8 = inp.tile([128, NDT, D], F8, tag="wv8")
                nc.scalar.dma_start(wv8, wv8_d[:, :, :])
                zt8 = inp.tile([128, NDT, 1024], F8, tag="zt8")
                nc.gpsimd.dma_start(zt8, zt8_d[:, :, :])

                # ---- V projection: j-tiles 0-1 bf16, 2-7 fp8 DoubleRow ----
                vsc = p1.enter_context(ExitStack())
                vps = vsc.enter_context(tc.tile_pool(name="v_ps", bufs=2, space="PSUM"))

                def v_sink(jt, vh, ps):
                    v8t = stg.tile([128, 1024], F8, tag="v8")
                    nc.scalar.activation(v8t, ps, Copy)
                    nc.sync.dma_start(
                        kv8_loc[jt // 4][jt % 4][
                            :, V_OFF + vh * 1024 : V_OFF + (vh + 1) * 1024
                        ],
                        v8t,
                    )
                    if jt < 2:  # bf16 pack: keys 0-255 only
                        vb = stg.tile([128, 1024], BF16, tag="vb")
                        nc.scalar.activation(vb, ps, Copy, scale=rk0_sc)
                        nc.sync.dma_start(
                            kvbf_loc[jt][
                                :, V_OFF + vh * 1024 : V_OFF + (vh + 1) * 1024
                            ],
                            vb,
                        )

                def v_bf16(vh, wvbf):
                    pss = [
                        vps.tile([128, 1024], F32, tag="vps", name=f"vbf{vh}_{jt}")
                        for jt in range(2)
                    ]
                    for dt in range(NDT):
                        for jt in range(2):
                            for c2 in range(2):
                                nc.tensor.matmul(
                                    pss[jt][:, c2 * 512 : (c2 + 1) * 512],
                                    zt[:, dt, jt * 128 : (jt + 1) * 128],
                                    wvbf[:, dt, c2 * 512 : (c2 + 1) * 512],
                                    start=(dt == 0),
                                    stop=(dt == NDT - 1),
                                )
                    for jt in range(2):
                        v_sink(jt, vh, pss[jt])

                def v_f8(jt, vh):
                    ps = vps.tile([128, 1024], F32, tag="vps")
                    for c2 in range(2):
                        for u in range(8):
                            nc.tensor.matmul(
                                ps[:, c2 * 512 : (c2 + 1) * 512],
                                zt8[:, 2 * u : 2 * u + 2, jt * 128 : (jt + 1) * 128],
                                wv8[
                                    :,
                                    2 * u : 2 * u + 2,
                                    vh * 1024 + c2 * 512 : vh * 1024 + (c2 + 1) * 512,
                                ],
                                start=(u == 0),
                                stop=(u == 7),
                                perf_mode=DR,
                            )
                    v_sink(jt, vh, ps)

                v_bf16(0, wvbf0)
                wvbf1 = wvbfp.tile([128, NDT, 1024], BF16, tag="wvbf", name="wvbf1")
                nc.scalar.dma_start(wvbf1, wvt_d[:, :, 1024:2048])
                for jt in (2, 3):
                    for vh in range(2):
                        v_f8(jt, vh)
                v_bf16(1, wvbf1)
                for jt in range(4):
                    nc.sync.dma_start(
                        kv8_loc[0][jt][:, ONES_OFF : ONES_OFF + 8], ones8
                    )
                for jt in range(2):
                    nc.sync.dma_start(
                        kvbf_loc[jt][:, ONES_OFF : ONES_OFF + 8], ones_bf
                    )
                nc.gpsimd.collective_compute(
                    "AllGather",
                    mybir.AluOpType.bypass,
                    replica_groups=groups,
                    ins=[kv8_loc[0].ap().opt()],
                    outs=[kv8_g[0].ap().opt()],
                )
                # xt loads for Q on the gpsimd ring (idle during phase 1)
                xtb = inp.tile([128, NDT, 128], BF16, tag="xtb")
                nc.gpsimd.dma_start(xtb, xtb_d[:, :, :])
                xt8 = inp.tile([128, NDT, 1024], F8, tag="xt8")
                nc.gpsimd.dma_start(xt8, xt8_d[:, :, :])
                for jt in range(4, 8):
                    for vh in range(2):
                        v_f8(jt, vh)
                    nc.sync.dma_start(
                        kv8_loc[1][jt - 4][:, ONES_OFF : ONES_OFF + 8], ones8
                    )
                nc.gpsimd.collective_compute(
                    "AllGather",
                    mybir.AluOpType.bypass,
                    replica_groups=groups,
                    ins=[kv8_loc[1].ap().opt()],
                    outs=[kv8_g[1].ap().opt()],
                )
                nc.gpsimd.collective_compute(
                    "AllReduce",
                    mybir.AluOpType.add,
                    replica_groups=groups,
                    ins=[kvbf_loc.ap().opt()],
                    outs=[kvbf_g.ap().opt()],
                )
                vsc.close()  # frees V PSUM for the Q-split pools

                # ---- Q projection: q-tile 0 (global rows < 1024) bf16;
                # q-tiles 1-7 (rows >= 1024) fp8 DoubleRow ----
                qxps = p1.enter_context(
                    tc.tile_pool(name="qx_ps", bufs=2, space="PSUM")
                )
                for t in range(NDT):
                    # Q panels on the sync ring: pack writes are done by now,
                    # and the scalar ring is degraded by AllGather traffic
                    wpt = wpp.tile([128, NDT, 128], BF16, tag="wp")
                    nc.sync.dma_start(wpt, wqp_d[t])
                    w8t = w8pp.tile([128, NDT, 128], F8, tag="w8p")
                    nc.sync.dma_start(w8t, wq8_d[t])
                    ps128 = qxps.tile([128, 128], F32, tag="ps128")
                    for dt in range(NDT):
                        nc.tensor.matmul(
                            ps128,
                            wpt[:, dt, :],
                            xtb[:, dt, :],
                            start=(dt == 0),
                            stop=(dt == NDT - 1),
                        )
                    psA = kqps.tile([128, 512], F32, tag="ps")
                    for u in range(8):
                        nc.tensor.matmul(
                            psA,
                            w8t[:, 2 * u : 2 * u + 2, :],
                            xt8[:, 2 * u : 2 * u + 2, 128:640],
                            start=(u == 0),
                            stop=(u == 7),
                            perf_mode=DR,
                        )
                    psB = qxps.tile([128, 384], F32, tag="psB")
                    for u in range(8):
                        nc.tensor.matmul(
                            psB,
                            w8t[:, 2 * u : 2 * u + 2, :],
                            xt8[:, 2 * u : 2 * u + 2, 640:1024],
                            start=(u == 0),
                            stop=(u == 7),
                            perf_mode=DR,
                        )
                    for ps, sl in (
                        (ps128, slice(0, 128)),
                        (psA, slice(128, 640)),
                        (psB, slice(640, 1024)),
                    ):
                        nc.scalar.activation(
                            qt[:, t, sl], ps, Ident, bias=bq_sb[:, t : t + 1]
                        )
                        nc.vector.tensor_scalar_add(
                            qt8[:, t, sl], ps, bq_sb[:, t : t + 1]
                        )

            # ---------------- Phase 2: causal attention ----------------
            with ExitStack() as p2:
                kv8p = p2.enter_context(tc.tile_pool(name="kv8", bufs=3))
                kvbfp = p2.enter_context(tc.tile_pool(name="kvbf", bufs=1))
                pt8p = p2.enter_context(tc.tile_pool(name="pt8", bufs=2))
                ptbfp = p2.enter_context(tc.tile_pool(name="ptbf", bufs=1))
                stp = p2.enter_context(tc.tile_pool(name="st_ps", bufs=3, space="PSUM"))
                pvp = p2.enter_context(tc.tile_pool(name="pv_ps", bufs=1, space="PSUM"))
                accp = p2.enter_context(tc.tile_pool(name="acc", bufs=1))
                fin = p2.enter_context(tc.tile_pool(name="fin", bufs=2))

                bv_bc = fin.tile([128, D], F32, tag="bv_bc")
                nc.gpsimd.dma_start(
                    bv_bc, bass.AP(tensor=bv_d, offset=0, ap=[[0, 128], [1, D]])
                )

                def epilogue(p, acc):
                    # acc layout: [V 0:1024 | den 1024:1032 | V 1032:2056]
                    rc = fin.tile([128, 1], F32, tag="rc")
                    nc.vector.reciprocal(rc, acc[:, 1024:1025])
                    of = fin.tile([128, D], F32, tag="of")
                    # out = acc/den + bv, chunked so DVE/DMA pipeline
                    for c2 in range(2):
                        sl = slice(c2 * 1024, (c2 + 1) * 1024)
                        asl = slice(c2 * 1032, c2 * 1032 + 1024)
                        nc.vector.scalar_tensor_tensor(
                            of[:, sl],
                            acc[:, asl],
                            rc,
                            bv_bc[:, sl],
                            mybir.AluOpType.mult,
                            mybir.AluOpType.add,
                        )
                        nc.scalar.dma_start(
                            out_d[p * 128 : (p + 1) * 128, sl], of[:, sl]
                        )

                for g in range(2):
                    p0 = 4 * g
                    acc = {
                        p: accp.tile(
                            [128, 2056], F32, tag=f"acc{p - p0}", name=f"acc{g}_{p}"
                        )
                        for p in range(p0, p0 + 4)
                    }
                    fresh = {p: [True, True] for p in range(p0, p0 + 4)}

                    def flush(p, chunk, pv):
                        # acc: [V 0:1024 | den 1024:1032] <- chunk 0,
                        #      [V 1032:2056] <- chunk 1
                        lo = 1032 * chunk
                        hi = lo + (1032 if chunk == 0 else 1024)
                        if fresh[p][chunk]:
                            nc.vector.tensor_copy(acc[p][:, lo:hi], pv)
                            fresh[p][chunk] = False
                        else:
                            nc.vector.tensor_add(acc[p][:, lo:hi], acc[p][:, lo:hi], pv)

                    def pv_mms(p, ph, W, pt, ks, wk0, is8):
                        # ks: k-tile indices of this window part; wk0: index of
                        # ks[0] within the W tile (bf16 W only holds 2 tiles)
                        off = 128 * (p - ph)
                        nk = len(ks)

                        def mms(pv, lo, c2s):
                            for c2 in c2s:
                                if is8:
                                    for u in range(nk // 2):
                                        nc.tensor.matmul(
                                            pv[
                                                :,
                                                (c2 - c2s[0]) * 512 : (c2 - c2s[0] + 1)
                                                * 512,
                                            ],
                                            pt[:, 2 * u : 2 * u + 2, off : off + 128],
                                            W[
                                                :,
                                                wk0 + 2 * u : wk0 + 2 * u + 2,
                                                V_OFF
                                                + c2 * 512 : V_OFF
                                                + (c2 + 1) * 512,
                                            ],
                                            start=(u == 0),
                                            stop=(u == nk // 2 - 1),
                                            perf_mode=DR,
                                        )
                                else:
                                    for i in range(nk):
                                        nc.tensor.matmul(
                                            pv[
                                                :,
                                                (c2 - c2s[0]) * 512 : (c2 - c2s[0] + 1)
                                                * 512,
                                            ],
                                            pt[:, i, off : off + 128],
                                            W[
                                                :,
                                                wk0 + i,
                                                V_OFF
                                                + c2 * 512 : V_OFF
                                                + (c2 + 1) * 512,
                                            ],
                                            start=(i == 0),
                                            stop=(i == nk - 1),
                                        )

                        # ones ride in pvA so the denominator lands with the
                        # first flush (epilogue reciprocal can start earlier)
                        pvA = pvp.tile([128, 1032], F32, tag="pvA")
                        mms(pvA, 0, (0, 1))
                        if is8:
                            for u in range(nk // 2):
                                nc.tensor.matmul(
                                    pvA[:, 1024:1032],
                                    pt[:, 2 * u : 2 * u + 2, off : off + 128],
                                    W[
                                        :,
                                        wk0 + 2 * u : wk0 + 2 * u + 2,
                                        ONES_OFF : ONES_OFF + 8,
                                    ],
                                    start=(u == 0),
                                    stop=(u == nk // 2 - 1),
                                    perf_mode=DR,
                                )
                        else:
                            for i in range(nk):
                                nc.tensor.matmul(
                                    pvA[:, 1024:1032],
                                    pt[:, i, off : off + 128],
                                    W[:, wk0 + i, ONES_OFF : ONES_OFF + 8],
                                    start=(i == 0),
                                    stop=(i == nk - 1),
                                )
                        flush(p, 0, pvA)
                        pvB = pvp.tile([128, 1024], F32, tag="pvB")
                        mms(pvB, 0, (2, 3))
                        flush(p, 1, pvB)

                    def window(h, r, is8, ks=(0, 1, 2, 3)):
                        ph = max(p0, r)
                        n = 128 * (p0 + 4 - ph)
                        nk = len(ks)
                        if is8:
                            W = kv8p.tile([128, 4, PACK], F8, tag="kv8")
                            nc.sync.dma_start(
                                W,
                                kv8_g[h][4 * r : 4 * r + 4].rearrange(
                                    "j p c -> p j c"
                                ),
                            )
                            pt = pt8p.tile([128, nk, n], F8, tag="pt8")
                            qsrc = qt8
                            wk0 = ks[0]
                        else:
                            assert h == 0 and r == 0 and ks == (0, 1)
                            W = kvbfp.tile([128, 2, PACK], BF16, tag="kvbf")
                            nc.gpsimd.dma_start(
                                W, kvbf_g[0:2].rearrange("j p c -> p j c")
                            )
                            pt = ptbfp.tile([128, nk, n], BF16, tag="ptbf")
                            qsrc = qt
                            wk0 = 0
                        for i, k in enumerate(ks):
                            st = stp.tile([128, n], F32, tag="st")
                            if is8 and n >= 256:
                                for u in range(8):
                                    nc.tensor.matmul(
                                        st,
                                        W[
                                            :, wk0 + i, 256 * u : 256 * (u + 1)
                                        ].rearrange("p (two f) -> p two f", two=2),
                                        qt8[
                                            :,
                                            2 * u : 2 * u + 2,
                                            128 * ph : 128 * ph + n,
                                        ],
                                        start=(u == 0),
                                        stop=(u == 7),
                                        perf_mode=DR,
                                    )
                            else:
                                for dt in range(NDT):
                                    nc.tensor.matmul(
                                        st,
                                        W[:, wk0 + i, dt * 128 : (dt + 1) * 128],
                                        qsrc[:, dt, 128 * ph : 128 * ph + n],
                                        start=(dt == 0),
                                        stop=(dt == NDT - 1),
                                    )
                            nc.scalar.activation(
                                pt[:, i, :], st, Exp, scale=SCALE, bias=nshift
                            )
                            if ph == r:
                                nc.vector.tensor_mul(
                                    pt[:, i, 0:128], pt[:, i, 0:128], msk[4 * h + k]
                                )
                        for p in range(ph, p0 + 4):
                            pv_mms(p, ph, W, pt, ks, wk0, is8)

                    if g == 0:
                        # fp8 windows first (AG-A lands earliest); the bf16
                        # part (keys 0-255) last, after the AllReduce
                        for r in range(1, 4):
                            window(0, r, is8=True)
                        for r in range(0, 4):
                            window(1, r, is8=True)
                        window(0, 0, is8=True, ks=(2, 3))
                        window(0, 0, is8=False, ks=(0, 1))
                        for p in range(p0, p0 + 4):
                            epilogue(p, acc[p])
                    else:
                        window(0, 1, is8=True)
                        window(0, 0, is8=True, ks=(2, 3))
                        window(0, 0, is8=False, ks=(0, 1))
                        for r in range(2, 8):
                            window(0, r, is8=True)
                        for r in range(0, 8):
                            window(1, r, is8=True)
                            if r >= p0:
                                epilogue(r, acc[r])

    nc.finalize()
    return nc


def make_in_maps(x, z, Wq, bq, Wk, bk, Wv, bv):
    bf = ml_dtypes.bfloat16
    f8 = ml_dtypes.float8_e4m3
    x = np.asarray(x, dtype=np.float32)
    z = np.asarray(z, dtype=np.float32)

    def tr_in(blk, dt):
        # [1024, 2048] -> [128 (d_low), 16 (dt), 1024 (row)]
        t = blk.T.astype(dt).reshape(NDT, 128, 1024).transpose(1, 0, 2)
        return np.ascontiguousarray(t)

    def w_panels(W, dt):
        # W[d, e]: -> [16 (t), 128 (d_low), 16 (dt), 128 (e_low)]
        t = W.astype(dt).reshape(NDT, 128, NDT, 128).transpose(2, 1, 0, 3)
        return np.ascontiguousarray(t)

    Wv = np.asarray(Wv, np.float32)
    wvt = np.ascontiguousarray(
        Wv.astype(bf).reshape(NDT, 128, D).transpose(1, 0, 2)
    )
    wv8 = np.ascontiguousarray(
        Wv.astype(f8).reshape(NDT, 128, D).transpose(1, 0, 2)
    )
    wkp = w_panels(np.asarray(Wk, np.float32), bf)
    wqp = w_panels(np.asarray(Wq, np.float32), bf)
    wq8 = w_panels(np.asarray(Wq, np.float32), f8)

    in_maps = []
    for c in range(NCORES):
        xtb_full = tr_in(x[c::8], bf)
        in_maps.append(
            {
                "xtb": np.ascontiguousarray(xtb_full[:, :, 0:128]),
                "xt8": tr_in(x[c::8], f8),
                "zt": tr_in(z[c * 1024 : (c + 1) * 1024], bf),
                "zt8": tr_in(z[c * 1024 : (c + 1) * 1024], f8),
                "wkp": wkp,
                "wqp": wqp,
                "wq8": wq8,
                "wvt": wvt,
                "wv8": wv8,
                "bq": np.asarray(bq, dtype=np.float32),
                "bk": np.asarray(bk, dtype=np.float32),
                "bv": np.asarray(bv, dtype=np.float32),
                "iu": (np.arange(128, dtype=np.float32) * 8 + c),
                "rk0": np.full(8, 1.0 if c == 0 else 0.0, dtype=np.float32),
            }
        )
    return in_maps


def kernel(x, z, Wq, bq, Wk, bk, Wv, bv):
    if "nc" not in _cache:
        t0 = time.time()
        _cache["nc"] = _build()
        _cache["build_s"] = time.time() - t0

    in_maps = make_in_maps(x, z, Wq, bq, Wk, bk, Wv, bv)

    t0 = time.time()
    last_err = None
    for attempt in range(3):
        try:
            res = run_bass_kernel_spmd(
                _cache["nc"], in_maps, core_ids=list(range(NCORES))
            )
            break
        except Exception as e:  # transient NRT_EXEC_UNIT_UNRECOVERABLE after a
            last_err = e  # prior process exits; an immediate retry succeeds
            time.sleep(10)
    else:
        raise last_err
    _cache["run_s"] = time.time() - t0

    full = np.empty((L, D), dtype=np.float32)
    for c in range(NCORES):
        full[c::8] = res.results[c]["out"]
    return full


# revision 44
# speedup vs baseline: 1.0443x; 1.0443x over previous
"""Causal self-attention (L=8192, D=2048) on 8 TRN2 NeuronCores.

Sharding: core c owns query rows x[c::8] (stride-8 interleave); KV rows
[c*1024, (c+1)*1024) are projected locally.  Local q-tile p (128 rows) covers
global rows [1024p + c, 1024p + 1016 + c], so causally it needs exactly KV
j-tiles 0..8p+7 - identical on every core (load-balanced static SPMD).

Precision: keys >= 1024 are consumed through fp8-e4m3 K/V/P with DoubleRow
matmuls (2x PE rate); keys < 1024 (where early rows' softmax is concentrated
and quantization noise would not average out) stay bf16.  Every rank packs its
K^T/V/ones j-tiles in fp8 ([K8 2048 | V8 2048 | ones 8 | pad] = 4112B/row) and
AllGathers them in two halves; rank 0's bf16 pack ([Kbf | Vbf | ones] x4112
bf16 cols) is broadcast via a rank-masked AllReduce(add).  exp is computed as
exp(s/sqrt(d) - 2.5) so P fits fp8 range; the shift cancels in num/den.

Host-side prep (free): x^T/z^T and all weight panels are pre-transposed and
pre-cast to bf16 in DMA-ready layouts, so phase 1 is pure projection matmuls.
A ~96-matmul warmup burst trips the PE HAM clock gate to 2.4 GHz before the
first projection.

Phase 1: warmup -> K proj -> V(j-tiles 0-3) -> AG8-A -> V(4-7) -> AG8-B + AR
-> Q proj (bf16 + fp8 sinks).  Phase 2 runs two q-group passes (q-tiles 0-3,
then 4-7) so only 4 f32 accumulators are SBUF-resident; within a pass, fp8
windows r>=1 run S^T (DoubleRow over dt pairs) -> exp -> P^T@[V|1] (DoubleRow
over k-tile pairs), and the two r=0 windows run the bf16 path from the
AllReduced pack.  Per-q-tile epilogue (scale by 1/den, +bv, DMA out) issues as
soon as that q-tile's last window is accumulated.
"""

import math
import time
from contextlib import ExitStack

import ml_dtypes
import numpy as np

import concourse.bass as bass
import concourse.tile as tile
from concourse import bacc, mybir
from concourse.bass_utils import run_bass_kernel_spmd

L = 8192
D = 2048  # d_x == d_attn == d_v
NCORES = 8
NDT = D // 128  # 16 contraction tiles
NQT = 8  # local 128-row q-tiles per core
PACK = 4112  # fp8: 2048 K | 2048 V | 8 ones | 8 pad ; bf16 pack same col count
V_OFF = 2048
ONES_OFF = 4096
SCALE = 1.0 / math.sqrt(D)
SHIFT = 2.5  # exp(s*SCALE - SHIFT): max p ~ e^3 = 20 << 240 (fp8e4 max)

F32 = mybir.dt.float32
BF16 = mybir.dt.bfloat16
F8 = mybir.dt.float8e4
DR = mybir.MatmulPerfMode.DoubleRow
Ident = mybir.ActivationFunctionType.Identity
Copy = mybir.ActivationFunctionType.Copy
Exp = mybir.ActivationFunctionType.Exp

_cache = {}


def _build():
    nc = bacc.Bacc("TRN2", num_devices=NCORES)

    ztb_d = nc.dram_tensor("ztb", [128, NDT, 256], BF16, kind="ExternalInput")
    zt8_d = nc.dram_tensor("zt8", [128, NDT, 1024], F8, kind="ExternalInput")
    xt8_d = nc.dram_tensor("xt8", [128, NDT, 1024], F8, kind="ExternalInput")
    xtb_d = nc.dram_tensor("xtb", [128, NDT, 128], BF16, kind="ExternalInput")
    wkp_d = nc.dram_tensor("wkp", [NDT, 128, NDT, 128], BF16, kind="ExternalInput")
    wk8_d = nc.dram_tensor("wk8", [NDT, 128, NDT, 128], F8, kind="ExternalInput")
    wqp_d = nc.dram_tensor("wqp", [NDT, 128, NDT, 128], BF16, kind="ExternalInput")
    wq8_d = nc.dram_tensor("wq8", [NDT, 128, NDT, 128], F8, kind="ExternalInput")
    wvt_d = nc.dram_tensor("wvt", [128, NDT, D], BF16, kind="ExternalInput")
    wv8_d = nc.dram_tensor("wv8", [128, NDT, D], F8, kind="ExternalInput")
    bq_d = nc.dram_tensor("bq", [D], F32, kind="ExternalInput")
    bk_d = nc.dram_tensor("bk", [D], F32, kind="ExternalInput")
    bv_d = nc.dram_tensor("bv", [D], F32, kind="ExternalInput")
    iu_d = nc.dram_tensor("iu", [128], F32, kind="ExternalInput")
    rk0_d = nc.dram_tensor("rk0", [8], F32, kind="ExternalInput")
    out_d = nc.dram_tensor("out", [1024, D], F32, kind="ExternalOutput")

    kv8_loc = [nc.dram_tensor(f"kv8loc{h}", [4, 128, PACK], F8) for h in range(2)]
    kv8_g = [
        nc.dram_tensor(f"kv8g{h}", [32, 128, PACK], F8, addr_space="Shared")
        for h in range(2)
    ]
    # bf16 pack only carries rank 0's keys 0-255 (j-tiles 0-1)
    kvbf_loc = nc.dram_tensor("kvbfloc", [2, 128, PACK], BF16)
    kvbf_g = nc.dram_tensor("kvbfg", [2, 128, PACK], BF16, addr_space="Shared")

    groups = [list(range(NCORES))]

    with tile.TileContext(nc) as tc:
        with ExitStack() as outer:
            cp = outer.enter_context(tc.tile_pool(name="consts", bufs=1))
            # jg[v, r] = 128*r + v
            jg = cp.tile([128, 8], F32, tag="jg")
            nc.gpsimd.iota(
                jg,
                pattern=[[128, 8]],
                base=0,
                channel_multiplier=1,
                allow_small_or_imprecise_dtypes=True,
            )
            # iu_bc[v, u] = 8*u + c (same for all partitions v)
            iu_bc = cp.tile([128, 128], F32, tag="iu_bc")
            nc.gpsimd.dma_start(
                iu_bc, bass.AP(tensor=iu_d, offset=0, ap=[[0, 128], [1, 128]])
            )
            # rk0f[v, j] = 1.0 iff this core is rank 0
            rk0f = cp.tile([128, 8], F32, tag="rk0f")
            nc.gpsimd.dma_start(
                rk0f, bass.AP(tensor=rk0_d, offset=0, ap=[[0, 128], [1, 8]])
            )
            rk0_sc = rk0f[:, 0:1]
            ones_bf = cp.tile([128, 8], BF16, tag="ones_bf")  # rk0-masked ones
            nc.vector.tensor_copy(ones_bf, rk0f)
            ones8 = cp.tile([128, 8], F8, tag="ones8")
            nc.vector.memset(ones8, 1.0)
            # msk[m][v, u] = (8u + c >= 128m + v): causal mask of diagonal tile
            msk = []
            for m in range(8):
                mt = cp.tile([128, 128], BF16, tag=f"msk{m}")
                nc.vector.tensor_scalar(
                    mt, iu_bc, jg[:, m : m + 1], None, mybir.AluOpType.is_ge
                )
                msk.append(mt)
            bq_sb = cp.tile([128, NDT], F32, tag="bq")
            nc.gpsimd.dma_start(
                bq_sb, bass.AP(tensor=bq_d, offset=0, ap=[[1, 128], [128, NDT]])
            )
            bk_sb = cp.tile([128, NDT], F32, tag="bk")
            nc.gpsimd.dma_start(
                bk_sb, bass.AP(tensor=bk_d, offset=0, ap=[[1, 128], [128, NDT]])
            )
            bkm_sb = cp.tile([128, NDT], F32, tag="bkm")  # rk0-masked K bias
            nc.vector.tensor_scalar_mul(bkm_sb, bk_sb, rk0_sc)
            nshift = cp.tile([128, 1], F32, tag="nshift")
            nc.vector.memset(nshift, -SHIFT)

            # qt (bf16, q-tile 0 only) / qt8 persist through all of phase 2
            qtp = outer.enter_context(tc.tile_pool(name="qt", bufs=1))
            qt = qtp.tile([128, NDT, 128], BF16, tag="qt")
            qt8 = qtp.tile([128, NDT, 1024], F8, tag="qt8")

            # ---- PE warmup: trip the HAM clock gate before real matmuls ----
            with ExitStack() as wm:
                wmp = wm.enter_context(tc.tile_pool(name="wm", bufs=1))
                wrm = wmp.tile([128, 128], BF16, tag="wrm")
                nc.vector.memset(wrm, 0.5)
                wps = wm.enter_context(tc.tile_pool(name="wm_ps", bufs=2, space="PSUM"))
                for _ in range(28):
                    wp_ps = wps.tile([128, 128], F32, tag="wps")
                    nc.tensor.matmul(wp_ps, wrm, wrm, start=True, stop=True)

            # ---------------- Phase 1: projections + collectives ----------------
            with ExitStack() as p1:
                ztp = p1.enter_context(tc.tile_pool(name="zt", bufs=1))
                ztb = ztp.tile([128, NDT, 256], BF16, tag="ztb")
                zt8 = ztp.tile([128, NDT, 1024], F8, tag="zt8")
                wvbfp = p1.enter_context(tc.tile_pool(name="wvbf", bufs=1))
                wpp = p1.enter_context(tc.tile_pool(name="wp", bufs=2))
                w8pp = p1.enter_context(tc.tile_pool(name="w8p", bufs=2))
                stg = p1.enter_context(tc.tile_pool(name="stg", bufs=2))
                inp = p1.enter_context(tc.tile_pool(name="inp", bufs=1))
                kqps = p1.enter_context(
                    tc.tile_pool(name="kq_ps", bufs=2, space="PSUM")
                )

                # critical first loads on the sync ring (first delivery ~3us;
                # the scalar HWDGE takes ~40us to first delivery)
                wk_pre = [
                    wpp.tile([128, NDT, 128], BF16, tag="wp", name=f"wk_pre{t}")
                    for t in range(2)
                ]
                wk8_pre = [
                    w8pp.tile([128, NDT, 128], F8, tag="w8p", name=f"wk8_pre{t}")
                    for t in range(2)
                ]
                for t in range(2):
                    nc.sync.dma_start(wk_pre[t], wkp_d[t])
                    nc.sync.dma_start(wk8_pre[t], wk8_d[t])
                nc.sync.dma_start(ztb, ztb_d[:, :, :])
                for ch in range(2):
                    nc.sync.dma_start(
                        zt8[:, 8 * ch : 8 * ch + 8, :], zt8_d[:, 8 * ch : 8 * ch + 8, :]
                    )

                # ---- K projection: j-cols 0:256 bf16, 256:1024 fp8 DR ----
                for t in range(NDT):
                    if t < 2:
                        wpt, w8t = wk_pre[t], wk8_pre[t]
                    else:
                        wpt = wpp.tile([128, NDT, 128], BF16, tag="wp")
                        w8t = w8pp.tile([128, NDT, 128], F8, tag="w8p")
                        ring = nc.sync if t < 5 else nc.scalar
                        ring.dma_start(wpt, wkp_d[t])
                        ring.dma_start(w8t, wk8_d[t])
                    psK = kqps.tile([128, 256], F32, tag="ps256")
                    for dt in range(NDT):
                        nc.tensor.matmul(
                            psK,
                            wpt[:, dt, :],
                            ztb[:, dt, :],
                            start=(dt == 0),
                            stop=(dt == NDT - 1),
                        )
                    psA = kqps.tile([128, 512], F32, tag="ps")
                    for u in range(8):
                        nc.tensor.matmul(
                            psA,
                            w8t[:, 2 * u : 2 * u + 2, :],
                            zt8[:, 2 * u : 2 * u + 2, 256:768],
                            start=(u == 0),
                            stop=(u == 7),
                            perf_mode=DR,
                        )
                    psB = kqps.tile([128, 256], F32, tag="ps256")
                    for u in range(8):
                        nc.tensor.matmul(
                            psB,
                            w8t[:, 2 * u : 2 * u + 2, :],
                            zt8[:, 2 * u : 2 * u + 2, 768:1024],
                            start=(u == 0),
                            stop=(u == 7),
                            perf_mode=DR,
                        )
                    # K8 pack: j 0:256 from psK, 256:768 psA, 768:1024 psB
                    k8a = stg.tile([128, 256], F8, tag="k8a")
                    nc.scalar.activation(k8a, psK, Ident, bias=bk_sb[:, t : t + 1])
                    for q in range(2):
                        nc.sync.dma_start(
                            kv8_loc[0][q][:, t * 128 : (t + 1) * 128],
                            k8a[:, q * 128 : (q + 1) * 128],
                        )
                    k8b = stg.tile([128, 512], F8, tag="k8b")
                    nc.scalar.activation(k8b, psA, Ident, bias=bk_sb[:, t : t + 1])
                    for j in range(4):  # global j-tiles 2..5
                        jt = 2 + j
                        nc.sync.dma_start(
                            kv8_loc[jt // 4][jt % 4][:, t * 128 : (t + 1) * 128],
                            k8b[:, j * 128 : (j + 1) * 128],
                        )
                    k8c = stg.tile([128, 256], F8, tag="k8c")
                    nc.scalar.activation(k8c, psB, Ident, bias=bk_sb[:, t : t + 1])
                    for j in range(2):  # global j-tiles 6..7
                        nc.sync.dma_start(
                            kv8_loc[1][2 + j][:, t * 128 : (t + 1) * 128],
                            k8c[:, j * 128 : (j + 1) * 128],
                        )
                    # bf16 pack: keys 0-255, rank 0 only
                    kb = stg.tile([128, 256], BF16, tag="kb")
                    nc.scalar.activation(
                        kb,
                        psK,
                        Ident,
                        bias=bkm_sb[:, t : t + 1],
                        scale=rk0_sc,
                    )
                    for q in range(2):
                        nc.sync.dma_start(
                            kvbf_loc[q][:, t * 128 : (t + 1) * 128],
                            kb[:, q * 128 : (q + 1) * 128],
                        )

                # bulk loads, ordered by first use (scalar ring drains in order)
                wvbf0 = wvbfp.tile([128, NDT, 1024], BF16, tag="wvbf", name="wvbf0")
                nc.scalar.dma_start(wvbf0, wvt_d[:, :, 0:1024])
                wv8 = inp.tile([128, NDT, D], F8, tag="wv8")
                nc.scalar.dma_start(wv8, wv8_d[:, :, :])


                # ---- V projection: j-tiles 0-1 bf16, 2-7 fp8 DoubleRow ----
                vsc = p1.enter_context(ExitStack())
                vps = vsc.enter_context(tc.tile_pool(name="v_ps", bufs=2, space="PSUM"))

                def v_sink(jt, vh, ps):
                    v8t = stg.tile([128, 1024], F8, tag="v8")
                    nc.scalar.activation(v8t, ps, Copy)
                    nc.sync.dma_start(
                        kv8_loc[jt // 4][jt % 4][
                            :, V_OFF + vh * 1024 : V_OFF + (vh + 1) * 1024
                        ],
                        v8t,
                    )
                    if jt < 2:  # bf16 pack: keys 0-255 only
                        vb = stg.tile([128, 1024], BF16, tag="vb")
                        nc.scalar.activation(vb, ps, Copy, scale=rk0_sc)
                        nc.sync.dma_start(
                            kvbf_loc[jt][
                                :, V_OFF + vh * 1024 : V_OFF + (vh + 1) * 1024
                            ],
                            vb,
                        )

                def v_bf16(vh, wvbf):
                    pss = [
                        vps.tile([128, 1024], F32, tag="vps", name=f"vbf{vh}_{jt}")
                        for jt in range(2)
                    ]
                    for dt in range(NDT):
                        for jt in range(2):
                            for c2 in range(2):
                                nc.tensor.matmul(
                                    pss[jt][:, c2 * 512 : (c2 + 1) * 512],
                                    ztb[:, dt, jt * 128 : (jt + 1) * 128],
                                    wvbf[:, dt, c2 * 512 : (c2 + 1) * 512],
                                    start=(dt == 0),
                                    stop=(dt == NDT - 1),
                                )
                    for jt in range(2):
                        v_sink(jt, vh, pss[jt])

                def v_f8(jt, vh):
                    ps = vps.tile([128, 1024], F32, tag="vps")
                    for c2 in range(2):
                        for u in range(8):
                            nc.tensor.matmul(
                                ps[:, c2 * 512 : (c2 + 1) * 512],
                                zt8[:, 2 * u : 2 * u + 2, jt * 128 : (jt + 1) * 128],
                                wv8[
                                    :,
                                    2 * u : 2 * u + 2,
                                    vh * 1024 + c2 * 512 : vh * 1024 + (c2 + 1) * 512,
                                ],
                                start=(u == 0),
                                stop=(u == 7),
                                perf_mode=DR,
                            )
                    v_sink(jt, vh, ps)

                v_bf16(0, wvbf0)
                wvbf1 = wvbfp.tile([128, NDT, 1024], BF16, tag="wvbf", name="wvbf1")
                nc.scalar.dma_start(wvbf1, wvt_d[:, :, 1024:2048])
                for jt in (2, 3):
                    for vh in range(2):
                        v_f8(jt, vh)
                v_bf16(1, wvbf1)
                for jt in range(4):
                    nc.sync.dma_start(
                        kv8_loc[0][jt][:, ONES_OFF : ONES_OFF + 8], ones8
                    )
                for jt in range(2):
                    nc.sync.dma_start(
                        kvbf_loc[jt][:, ONES_OFF : ONES_OFF + 8], ones_bf
                    )
                nc.gpsimd.collective_compute(
                    "AllGather",
                    mybir.AluOpType.bypass,
                    replica_groups=groups,
                    ins=[kv8_loc[0].ap().opt()],
                    outs=[kv8_g[0].ap().opt()],
                )
                # xt loads for Q on the gpsimd ring (idle during phase 1)
                xtb = inp.tile([128, NDT, 128], BF16, tag="xtb")
                nc.gpsimd.dma_start(xtb, xtb_d[:, :, :])
                xt8 = inp.tile([128, NDT, 1024], F8, tag="xt8")
                nc.gpsimd.dma_start(xt8, xt8_d[:, :, :])
                for jt in range(4, 8):
                    for vh in range(2):
                        v_f8(jt, vh)
                    nc.sync.dma_start(
                        kv8_loc[1][jt - 4][:, ONES_OFF : ONES_OFF + 8], ones8
                    )
                nc.gpsimd.collective_compute(
                    "AllGather",
                    mybir.AluOpType.bypass,
                    replica_groups=groups,
                    ins=[kv8_loc[1].ap().opt()],
                    outs=[kv8_g[1].ap().opt()],
                )
                nc.gpsimd.collective_compute(
                    "AllReduce",
                    mybir.AluOpType.add,
                    replica_groups=groups,
                    ins=[kvbf_loc.ap().opt()],
                    outs=[kvbf_g.ap().opt()],
                )
                vsc.close()  # frees V PSUM for the Q-split pools

                # ---- Q projection: q-tile 0 (global rows < 1024) bf16;
                # q-tiles 1-7 (rows >= 1024) fp8 DoubleRow ----
                qxps = p1.enter_context(
                    tc.tile_pool(name="qx_ps", bufs=2, space="PSUM")
                )
                for t in range(NDT):
                    # Q panels on the sync ring: pack writes are done by now,
                    # and the scalar ring is degraded by AllGather traffic
                    wpt = wpp.tile([128, NDT, 128], BF16, tag="wp")
                    nc.sync.dma_start(wpt, wqp_d[t])
                    w8t = w8pp.tile([128, NDT, 128], F8, tag="w8p")
                    nc.sync.dma_start(w8t, wq8_d[t])
                    ps128 = qxps.tile([128, 128], F32, tag="ps128")
                    for dt in range(NDT):
                        nc.tensor.matmul(
                            ps128,
                            wpt[:, dt, :],
                            xtb[:, dt, :],
                            start=(dt == 0),
                            stop=(dt == NDT - 1),
                        )
                    psA = kqps.tile([128, 512], F32, tag="ps")
                    for u in range(8):
                        nc.tensor.matmul(
                            psA,
                            w8t[:, 2 * u : 2 * u + 2, :],
                            xt8[:, 2 * u : 2 * u + 2, 128:640],
                            start=(u == 0),
                            stop=(u == 7),
                            perf_mode=DR,
                        )
                    psB = qxps.tile([128, 384], F32, tag="psB")
                    for u in range(8):
                        nc.tensor.matmul(
                            psB,
                            w8t[:, 2 * u : 2 * u + 2, :],
                            xt8[:, 2 * u : 2 * u + 2, 640:1024],
                            start=(u == 0),
                            stop=(u == 7),
                            perf_mode=DR,
                        )
                    # bf16 qt only holds q-tile 0 (the only bf16 S consumer)
                    nc.scalar.activation(
                        qt[:, t, :], ps128, Ident, bias=bq_sb[:, t : t + 1]
                    )
                    for ps, sl in (
                        (ps128, slice(0, 128)),
                        (psA, slice(128, 640)),
                        (psB, slice(640, 1024)),
                    ):
                        nc.vector.tensor_scalar_add(
                            qt8[:, t, sl], ps, bq_sb[:, t : t + 1]
                        )

            # ---------------- Phase 2: causal attention ----------------
            with ExitStack() as p2:
                kv8p = p2.enter_context(tc.tile_pool(name="kv8", bufs=4))
                kvbfp = p2.enter_context(tc.tile_pool(name="kvbf", bufs=1))
                pt8p = p2.enter_context(tc.tile_pool(name="pt8", bufs=2))
                ptbfp = p2.enter_context(tc.tile_pool(name="ptbf", bufs=1))
                stp = p2.enter_context(tc.tile_pool(name="st_ps", bufs=3, space="PSUM"))
                pvp = p2.enter_context(tc.tile_pool(name="pv_ps", bufs=1, space="PSUM"))
                accp = p2.enter_context(tc.tile_pool(name="acc", bufs=1))
                fin = p2.enter_context(tc.tile_pool(name="fin", bufs=2))

                bv_bc = fin.tile([128, D], F32, tag="bv_bc")
                nc.gpsimd.dma_start(
                    bv_bc, bass.AP(tensor=bv_d, offset=0, ap=[[0, 128], [1, D]])
                )

                def epilogue(p, acc):
                    # acc layout: [V 0:1024 | den 1024:1032 | V 1032:2056]
                    rc = fin.tile([128, 1], F32, tag="rc")
                    nc.vector.reciprocal(rc, acc[:, 1024:1025])
                    of = fin.tile([128, D], F32, tag="of")
                    # out = acc/den + bv, chunked so DVE/DMA pipeline
                    for c2 in range(2):
                        sl = slice(c2 * 1024, (c2 + 1) * 1024)
                        asl = slice(c2 * 1032, c2 * 1032 + 1024)
                        nc.vector.scalar_tensor_tensor(
                            of[:, sl],
                            acc[:, asl],
                            rc,
                            bv_bc[:, sl],
                            mybir.AluOpType.mult,
                            mybir.AluOpType.add,
                        )
                        nc.scalar.dma_start(
                            out_d[p * 128 : (p + 1) * 128, sl], of[:, sl]
                        )

                for g in range(2):
                    p0 = 4 * g
                    acc = {
                        p: accp.tile(
                            [128, 2056], F32, tag=f"acc{p - p0}", name=f"acc{g}_{p}"
                        )
                        for p in range(p0, p0 + 4)
                    }
                    fresh = {p: [True, True] for p in range(p0, p0 + 4)}

                    def flush(p, chunk, pv):
                        # acc: [V 0:1024 | den 1024:1032] <- chunk 0,
                        #      [V 1032:2056] <- chunk 1
                        lo = 1032 * chunk
                        hi = lo + (1032 if chunk == 0 else 1024)
                        if fresh[p][chunk]:
                            nc.vector.tensor_copy(acc[p][:, lo:hi], pv)
                            fresh[p][chunk] = False
                        else:
                            nc.vector.tensor_add(acc[p][:, lo:hi], acc[p][:, lo:hi], pv)

                    def pv_mms(p, ph, W, pt, ks, wk0, is8):
                        # ks: k-tile indices of this window part; wk0: index of
                        # ks[0] within the W tile (bf16 W only holds 2 tiles)
                        off = 128 * (p - ph)
                        nk = len(ks)

                        def mms(pv, lo, c2s):
                            for c2 in c2s:
                                if is8:
                                    for u in range(nk // 2):
                                        nc.tensor.matmul(
                                            pv[
                                                :,
                                                (c2 - c2s[0]) * 512 : (c2 - c2s[0] + 1)
                                                * 512,
                                            ],
                                            pt[:, 2 * u : 2 * u + 2, off : off + 128],
                                            W[
                                                :,
                                                wk0 + 2 * u : wk0 + 2 * u + 2,
                                                V_OFF
                                                + c2 * 512 : V_OFF
                                                + (c2 + 1) * 512,
                                            ],
                                            start=(u == 0),
                                            stop=(u == nk // 2 - 1),
                                            perf_mode=DR,
                                        )
                                else:
                                    for i in range(nk):
                                        nc.tensor.matmul(
                                            pv[
                                                :,
                                                (c2 - c2s[0]) * 512 : (c2 - c2s[0] + 1)
                                                * 512,
                                            ],
                                            pt[:, i, off : off + 128],
                                            W[
                                                :,
                                                wk0 + i,
                                                V_OFF
                                                + c2 * 512 : V_OFF
                                                + (c2 + 1) * 512,
                                            ],
                                            start=(i == 0),
                                            stop=(i == nk - 1),
                                        )

                        # ones ride in pvA so the denominator lands with the
                        # first flush (epilogue reciprocal can start earlier)
                        pvA = pvp.tile([128, 1032], F32, tag="pvA")
                        mms(pvA, 0, (0, 1))
                        if is8:
                            for u in range(nk // 2):
                                nc.tensor.matmul(
                                    pvA[:, 1024:1032],
                                    pt[:, 2 * u : 2 * u + 2, off : off + 128],
                                    W[
                                        :,
                                        wk0 + 2 * u : wk0 + 2 * u + 2,
                                        ONES_OFF : ONES_OFF + 8,
                                    ],
                                    start=(u == 0),
                                    stop=(u == nk // 2 - 1),
                                    perf_mode=DR,
                                )
                        else:
                            for i in range(nk):
                                nc.tensor.matmul(
                                    pvA[:, 1024:1032],
                                    pt[:, i, off : off + 128],
                                    W[:, wk0 + i, ONES_OFF : ONES_OFF + 8],
                                    start=(i == 0),
                                    stop=(i == nk - 1),
                                )
                        flush(p, 0, pvA)
                        pvB = pvp.tile([128, 1024], F32, tag="pvB")
                        mms(pvB, 0, (2, 3))
                        flush(p, 1, pvB)

                    def window(h, r, is8=True, ks=(0, 1, 2, 3), W=None,
                               ph_ov=None, n_ov=None):
                        ph = max(p0, r) if ph_ov is None else ph_ov
                        n = 128 * (p0 + 4 - ph) if n_ov is None else n_ov
                        nk = len(ks)
                        if is8:
                            if W is None:
                                W = kv8p.tile([128, 4, PACK], F8, tag="kv8")
                                nc.sync.dma_start(
                                    W,
                                    kv8_g[h][4 * r : 4 * r + 4].rearrange(
                                        "j p c -> p j c"
                                    ),
                                )
                            pt = pt8p.tile([128, nk, n], F8, tag="pt8")
                            qsrc = qt8
                            wk0 = ks[0]
                        else:
                            assert h == 0 and r == 0 and ks == (0, 1) and ph == 0
                            W = kvbfp.tile([128, 2, PACK], BF16, tag="kvbf")
                            nc.gpsimd.dma_start(
                                W, kvbf_g[0:2].rearrange("j p c -> p j c")
                            )
                            pt = ptbfp.tile([128, nk, n], BF16, tag="ptbf")
                            qsrc = qt
                            wk0 = 0
                        for i, k in enumerate(ks):
                            st = stp.tile([128, n], F32, tag="st")
                            if is8 and n >= 256:
                                for u in range(8):
                                    nc.tensor.matmul(
                                        st,
                                        W[
                                            :, wk0 + i, 256 * u : 256 * (u + 1)
                                        ].rearrange("p (two f) -> p two f", two=2),
                                        qt8[
                                            :,
                                            2 * u : 2 * u + 2,
                                            128 * ph : 128 * ph + n,
                                        ],
                                        start=(u == 0),
                                        stop=(u == 7),
                                        perf_mode=DR,
                                    )
                            else:
                                for dt in range(NDT):
                                    nc.tensor.matmul(
                                        st,
                                        W[:, wk0 + i, dt * 128 : (dt + 1) * 128],
                                        qsrc[:, dt, 128 * ph : 128 * ph + n],
                                        start=(dt == 0),
                                        stop=(dt == NDT - 1),
                                    )
                            nc.scalar.activation(
                                pt[:, i, :], st, Exp, scale=SCALE, bias=nshift
                            )
                            if ph == r:
                                nc.vector.tensor_mul(
                                    pt[:, i, 0:128], pt[:, i, 0:128], msk[4 * h + k]
                                )
                        for p in range(ph, ph + n // 128):
                            pv_mms(p, ph, W, pt, ks, wk0, is8)
                        return W

                    if g == 0:
                        # fp8 windows first (AG-A lands earliest); window
                        # (0,0)'s bf16 part (q-tile 0 x keys 0-255) last,
                        # after the AllReduce
                        for r in range(1, 4):
                            window(0, r)
                        for r in range(0, 4):
                            window(1, r)
                        W00 = window(0, 0, ph_ov=1)  # tiles 1-3, all 4 k
                        window(0, 0, ks=(2, 3), W=W00, ph_ov=0, n_ov=128)
                        window(0, 0, is8=False, ks=(0, 1), ph_ov=0, n_ov=128)
                        for p in range(p0, p0 + 4):
                            epilogue(p, acc[p])
                    else:
                        # pass g1 is fully fp8 (rows >= 4096 are all diffuse)
                        for r in range(0, 8):
                            window(0, r)
                        for r in range(0, 8):
                            window(1, r)
                            if r >= p0:
                                epilogue(r, acc[r])

    nc.finalize()
    return nc


def make_in_maps(x, z, Wq, bq, Wk, bk, Wv, bv):
    bf = ml_dtypes.bfloat16
    f8 = ml_dtypes.float8_e4m3
    x = np.asarray(x, dtype=np.float32)
    z = np.asarray(z, dtype=np.float32)

    def tr_in(blk, dt):
        # [1024, 2048] -> [128 (d_low), 16 (dt), 1024 (row)]
        t = blk.T.astype(dt).reshape(NDT, 128, 1024).transpose(1, 0, 2)
        return np.ascontiguousarray(t)

    def w_panels(W, dt):
        # W[d, e]: -> [16 (t), 128 (d_low), 16 (dt), 128 (e_low)]
        t = W.astype(dt).reshape(NDT, 128, NDT, 128).transpose(2, 1, 0, 3)
        return np.ascontiguousarray(t)

    Wv = np.asarray(Wv, np.float32)
    wvt = np.ascontiguousarray(
        Wv.astype(bf).reshape(NDT, 128, D).transpose(1, 0, 2)
    )
    wv8 = np.ascontiguousarray(
        Wv.astype(f8).reshape(NDT, 128, D).transpose(1, 0, 2)
    )
    wkp = w_panels(np.asarray(Wk, np.float32), bf)
    wk8 = w_panels(np.asarray(Wk, np.float32), f8)
    wqp = w_panels(np.asarray(Wq, np.float32), bf)
    wq8 = w_panels(np.asarray(Wq, np.float32), f8)

    in_maps = []
    for c in range(NCORES):
        xtb_full = tr_in(x[c::8], bf)
        ztb_full = tr_in(z[c * 1024 : (c + 1) * 1024], bf)
        in_maps.append(
            {
                "xtb": np.ascontiguousarray(xtb_full[:, :, 0:128]),
                "xt8": tr_in(x[c::8], f8),
                "ztb": np.ascontiguousarray(ztb_full[:, :, 0:256]),
                "zt8": tr_in(z[c * 1024 : (c + 1) * 1024], f8),
                "wkp": wkp,
                "wk8": wk8,
                "wqp": wqp,
                "wq8": wq8,
                "wvt": wvt,
                "wv8": wv8,
                "bq": np.asarray(bq, dtype=np.float32),
                "bk": np.asarray(bk, dtype=np.float32),
                "bv": np.asarray(bv, dtype=np.float32),
                "iu": (np.arange(128, dtype=np.float32) * 8 + c),
                "rk0": np.full(8, 1.0 if c == 0 else 0.0, dtype=np.float32),
            }
        )
    return in_maps


def kernel(x, z, Wq, bq, Wk, bk, Wv, bv):
    if "nc" not in _cache:
        t0 = time.time()
        _cache["nc"] = _build()
        _cache["build_s"] = time.time() - t0

    in_maps = make_in_maps(x, z, Wq, bq, Wk, bk, Wv, bv)

    t0 = time.time()
    last_err = None
    for attempt in range(3):
        try:
            res = run_bass_kernel_spmd(
                _cache["nc"], in_maps, core_ids=list(range(NCORES))
            )
            break
        except Exception as e:  # transient NRT_EXEC_UNIT_UNRECOVERABLE after a
            last_err = e  # prior process exits; an immediate retry succeeds
            time.sleep(10)
    else:
        raise last_err
    _cache["run_s"] = time.time() - t0

    full = np.empty((L, D), dtype=np.float32)
    for c in range(NCORES):
        full[c::8] = res.results[c]["out"]
    return full
